# revision 1
# baseline (speedup 1.0000x reference)
"""ConvSTFT kernel for Trainium2 (Bass/Tile), data-parallel over batch on 8 cores.

Math: out[b, k, f, i] = sum_n xp[b, 320 f + n] * basis[i*513 + k, n]
where xp = x padded with 512 zeros on both sides, f in [0, 501), n in [0, 1024).

Key layout trick: let Xs[p, j] = xp[p + 64 j] (p in [0,128)).  Since
320 = 64*5 and 128 = 64*2, the contraction chunk c (n = 128 c + p) of
frame f reads Xs[p, 2c + 5f] — i.e. every matmul rhs is a stride-5
column view of Xs.  Xs is built on-chip with PE transposes of
overlapping 128-sample windows (hop 64) staged by a single DMA.
No im2col materialization is ever needed.

Matmuls run in float32r (TF32-like) which streams at 1 col/cycle vs
plain fp32's 1/4 rate; accumulation stays fp32 in PSUM.
"""

import numpy as np
from contextlib import ExitStack

import concourse.bass as bass
import concourse.tile as tile
from concourse import bacc, mybir

# problem constants (hardcoded per harness contract)
B, T = 32, 160000
NCORES = 8
BPC = B // NCORES          # batches per core
HOP, NFFT = 320, 1024
BINS, F = 513, 501         # freq bins, frames
FP = 502                   # frames padded to even (fp32r needs even N)
K2 = 2 * BINS              # 1026 basis rows
PAD = NFFT // 2            # 512
NT = 20                    # transpose tiles per batch
JC = NT * 128              # 2560 Xs columns
L = 8192 * (NT - 1) + 64 * 127 + 128   # 163904 padded xp length
MM_DT = mybir.dt.float32r

_STATE: dict = {}


def _build_nc():
    nc = bacc.Bacc(
        "TRN2", target_bir_lowering=False, debug=False, num_devices=NCORES
    )
    f32 = mybir.dt.float32
    xp = nc.dram_tensor("xp", [BPC, L], f32, kind="ExternalInput").ap()
    bt = nc.dram_tensor("bt", [128, 8 * K2], f32, kind="ExternalInput").ap()
    ident = nc.dram_tensor("ident", [128, 128], f32, kind="ExternalInput").ap()
    out = nc.dram_tensor("out", [BPC, BINS, F, 2], f32, kind="ExternalOutput").ap()

    with tile.TileContext(nc) as tc, ExitStack() as ctx:
        const_pool = ctx.enter_context(tc.tile_pool(name="const", bufs=1))
        stag_pool = ctx.enter_context(tc.tile_pool(name="stag", bufs=2))
        xs_pool = ctx.enter_context(tc.tile_pool(name="xs", bufs=2))
        st_pool = ctx.enter_context(tc.tile_pool(name="st", bufs=3))
        tp_pool = ctx.enter_context(tc.tile_pool(name="tp", bufs=2, space="PSUM"))
        acc_pool = ctx.enter_context(tc.tile_pool(name="acc", bufs=2, space="PSUM"))
        px_pool = ctx.enter_context(tc.tile_pool(name="px", bufs=1, space="PSUM"))

        ident_sb = const_pool.tile([128, 128], f32, tag="ident")
        nc.sync.dma_start(ident_sb[:], ident)
        bt_sb = const_pool.tile([128, 8 * K2], MM_DT, tag="bt")
        nc.sync.dma_start(bt_sb[:], bt.bitcast(MM_DT))

        for b in range(BPC):
            # stage overlapping windows: stag[k, 128 t + r] = xp[b, 8192 t + 64 k + r]
            stag = stag_pool.tile([128, JC], f32, tag="stag")
            src = bass.AP(xp.tensor, b * L, [[64, 128], [8192, NT], [1, 128]])
            dst = stag[:].rearrange("k (t r) -> k t r", r=128)
            nc.sync.dma_start(dst, src)

            # transpose to Xs[p, j] = xp[b, p + 64 j]; stored as fp32r
            # (the PSUM->SBUF copy performs the fp32r rounding)
            xs = xs_pool.tile([128, JC], MM_DT, tag="xs")
            for t in range(NT):
                pt = tp_pool.tile([128, 128], f32, tag="tp")
                nc.tensor.transpose(
                    pt[:], stag[:, t * 128:(t + 1) * 128], ident_sb[:]
                )
                nc.vector.tensor_copy(xs[:, t * 128:(t + 1) * 128], pt[:])

            # main matmuls: 4 (real,imag) chunk pairs of 128 rows
            for r in range(4):
                ps_r = acc_pool.tile([128, FP], f32, tag="accr")
                ps_i = acc_pool.tile([128, FP], f32, tag="acci")
                for c in range(8):
                    rhs = xs[:, 2 * c: 2 * c + 5 * FP: 5]
                    lr = bt_sb[:, c * K2 + 128 * r: c * K2 + 128 * r + 128]
                    li = bt_sb[:, c * K2 + 512 + 128 * r: c * K2 + 512 + 128 * r + 128]
                    nc.tensor.matmul(
                        ps_r[:], lr, rhs,
                        start=(c == 0), stop=(c == 7),
                    )
                    nc.tensor.matmul(
                        ps_i[:], li, rhs,
                        start=(c == 0), stop=(c == 7),
                    )
                st = st_pool.tile([128, 2 * F], f32, tag="st")
                nc.vector.tensor_copy(st[:, 0:2 * F:2], ps_r[:, 0:F])
                nc.vector.tensor_copy(st[:, 1:2 * F:2], ps_i[:, 0:F])
                nc.sync.dma_start(out[b, 128 * r:128 * r + 128, :, :], st[:])

            # last row pair: real bin 512 (col 1024) + imag bin 512 (col 1025)
            ps_x = px_pool.tile([2, FP], f32, tag="px")
            for c in range(8):
                rhs = xs[:, 2 * c: 2 * c + 5 * FP: 5]
                lx = bt_sb[:, c * K2 + 1024: c * K2 + 1026]
                nc.tensor.matmul(
                    ps_x[:], lx, rhs,
                    start=(c == 0), stop=(c == 7),
                )
            st_x = st_pool.tile([2, F], f32, tag="stx")
            nc.vector.tensor_copy(st_x[:], ps_x[:, 0:F])
            nc.sync.dma_start(out[b, 512:513, :, 0:1], st_x[0:1, :])
            nc.sync.dma_start(out[b, 512:513, :, 1:2], st_x[1:2, :])

    nc.compile()
    return nc


def _host_prep_basis(basis: np.ndarray):
    # reorder rows so chunks are [real 0:512 | imag 0:512 | real512, imag512]
    order = np.concatenate(
        [np.arange(0, 512), np.arange(513, 1025), [512], [1025]]
    )
    bt = basis[order].T.astype(np.float32)          # [1024, 1026]
    bt_sb = np.ascontiguousarray(
        bt.reshape(8, 128, K2).transpose(1, 0, 2).reshape(128, 8 * K2)
    )
    return bt_sb


def _get_exec():
    """Build (once) and return a cached executor fn(in_maps) -> full output."""
    if "exec" in _STATE:
        return _STATE["exec"]

    from concourse import bass2jax

    nc = _build_nc()

    def run(in_maps):
        res = bass2jax.run_bass_via_pjrt(nc, in_maps, n_cores=NCORES)
        return np.concatenate([r["out"] for r in res], axis=0)

    _STATE["exec"] = run
    return run


def _prep_inputs(x: np.ndarray, basis: np.ndarray):
    xp_all = np.zeros((B, L), np.float32)
    xp_all[:, PAD:PAD + T] = np.asarray(x, np.float32)
    bt_sb = _host_prep_basis(np.asarray(basis, np.float32))
    ident = np.eye(128, dtype=np.float32)
    in_maps = [
        {
            "xp": xp_all[BPC * c:BPC * (c + 1)],
            "bt": bt_sb,
            "ident": ident,
        }
        for c in range(NCORES)
    ]
    return in_maps


def kernel(x: np.ndarray, basis: np.ndarray) -> np.ndarray:
    run = _get_exec()
    in_maps = _prep_inputs(x, basis)
    return run(in_maps)                            # [32, 513, 501, 2]



# revision 16
# speedup vs baseline: 1.5981x; 1.5981x over previous
"""ConvSTFT kernel for Trainium2 (Bass/Tile), data-parallel over batch on 8 cores.

Math: out[b, k, f, i] = sum_n xp[b, 320 f + n] * basis[i*513 + k, n]
where xp = x padded with 512 zeros on both sides, f in [0, 501), n in [0, 1024).

Layout trick: Xs[p, j] = xp[p + 64 j] (p in [0,128)).  Since 320 = 64*5 and
128 = 64*2, contraction chunk c (n = 128 c + p) of frame f reads
Xs[p, 2c + 5f] — every matmul rhs is a stride-5 column view of Xs.  Xs is
built straight from DRAM by the DMA XBAR transpose (16x128 tiles), so no
staging buffer, no PE transposes, no PSUM round-trip for the input.

Everything runs in bf16 (inputs, basis, output) with fp32 PSUM accumulation;
the relative-error budget (2e-2) is ~5x above the ~4e-3 this costs.  The
output tensor is written bf16 and widened to fp32 on the host.

Basis rows are repacked to exactly 1024 = 8x128 non-zero rows: imag bins 0
and 512 are identically zero (sin(0), sin(pi n)); real bin 512 rides in the
imag-tile-0 slot at partition 0 and is routed to its own output row with a
tiny fixup, so there is no extra short matmul group.
"""

import numpy as np
from contextlib import ExitStack

import concourse.bass as bass
import concourse.tile as tile
from concourse import bacc, mybir

# problem constants (hardcoded per harness contract)
B, T = 32, 160000
NCORES = 8
BPC = B // NCORES          # batches per core
HOP, NFFT = 320, 1024
BINS, F = 513, 501         # freq bins, frames
FP = 502                   # frames padded to even
PAD = NFFT // 2            # 512
JC = 2560                  # Xs columns (>= 2*7 + 5*(FP-1) + 1, mult of 16)
L = 127 + 64 * (JC - 1) + 1  # 163904 padded xp length
BF16 = mybir.dt.bfloat16

_STATE: dict = {}


def _build_nc():
    nc = bacc.Bacc(
        "TRN2", target_bir_lowering=False, debug=False, num_devices=NCORES
    )
    f32 = mybir.dt.float32
    xp = nc.dram_tensor("xp", [BPC, L], BF16, kind="ExternalInput").ap()
    bt = nc.dram_tensor("bt", [128, 8 * 1024], BF16, kind="ExternalInput").ap()
    out = nc.dram_tensor("out", [BPC, BINS, F, 2], BF16, kind="ExternalOutput").ap()

    with tile.TileContext(nc) as tc, ExitStack() as ctx:
        const_pool = ctx.enter_context(tc.tile_pool(name="const", bufs=1))
        xs_pool = ctx.enter_context(tc.tile_pool(name="xs", bufs=2))
        st_pool = ctx.enter_context(tc.tile_pool(name="st", bufs=3))
        sx_pool = ctx.enter_context(tc.tile_pool(name="sx", bufs=2))
        acc_pool = ctx.enter_context(tc.tile_pool(name="acc", bufs=2, space="PSUM"))

        # warm the PE p-state during the startup DMA window: dummy matmuls
        # keep the tensor engine's busy-streak alive so the first real
        # matmuls run at full clock instead of the cold 0.65 GHz p-state
        wu_pool = ctx.enter_context(tc.tile_pool(name="wu", bufs=1))
        wup_pool = ctx.enter_context(tc.tile_pool(name="wup", bufs=1, space="PSUM"))
        dummy = wu_pool.tile([128, 512], BF16, tag="dummy")
        nc.vector.memset(dummy[:], 0)
        pw = wup_pool.tile([128, 512], f32, tag="pw")
        for _ in range(28):
            nc.tensor.matmul(pw[:], dummy[:, 0:128], dummy[:], start=True, stop=True)

        # Startup DMA chain (each DMA waits its scheduled predecessor, ~2us
        # per hop): order it [wt r0, xbar b0, wt r1, r2, r3] so the first
        # matmul group can start ~8.5us in, right as the PE warmup ends.
        bt_sb = const_pool.tile([128, 8 * 1024], BF16, tag="bt")
        nc.scalar.dma_start(bt_sb[:, 0:2048], bt[:, 0:2048])

        xs0 = xs_pool.tile([128, JC], BF16, tag="xs")
        src0 = bass.AP(xp.tensor, 0, [[64, JC], [1, 128]])
        nc.sync.dma_start_transpose(xs0[:], src0)

        wt_last = None
        for g in range(1, 4):
            wt_last = nc.scalar.dma_start(
                bt_sb[:, g * 2048:(g + 1) * 2048], bt[:, g * 2048:(g + 1) * 2048]
            )

        for b in range(BPC):
            # Xs[p, j] = xp[b, p + 64 j] via DMA XBAR transpose from DRAM
            if b == 0:
                xs = xs0
            else:
                xs = xs_pool.tile([128, JC], BF16, tag="xs")
                src = bass.AP(xp.tensor, b * L, [[64, JC], [1, 128]])
                xbi = nc.sync.dma_start_transpose(xs[:], src)
                if b == 1:
                    # keep this prefetch behind the last basis load in the
                    # serialized DMA chain: r2/r3 weights gate batch-0 compute
                    import bass_rust as _br
                    xbi.ins.add_dependency(
                        wt_last.ins.name, _br.DependencyInfo(sync=True, no_sync=False)
                    )

            for r in range(4):
                ps_r = acc_pool.tile([128, FP], f32, tag="accr")
                ps_i = acc_pool.tile([128, FP], f32, tag="acci")
                for c in range(8):
                    rhs = xs[:, 2 * c: 2 * c + 5 * FP: 5]
                    lr = bt_sb[:, r * 2048 + c * 256: r * 2048 + c * 256 + 128]
                    li = bt_sb[:, r * 2048 + c * 256 + 128: r * 2048 + c * 256 + 256]
                    nc.tensor.matmul(ps_r[:], lr, rhs, start=(c == 0), stop=(c == 7))
                    nc.tensor.matmul(ps_i[:], li, rhs, start=(c == 0), stop=(c == 7))

                # interleave (re, im) into bf16 st; DVE takes re, Act takes im
                st = st_pool.tile([128, 2 * FP], BF16, tag="st")
                nc.vector.tensor_copy(st[:, 0:2 * FP:2], ps_r[:])
                nc.scalar.copy(st[:, 1:2 * FP:2], ps_i[:])
                if r == 0:
                    # partition 0 of ps_i holds real bin 512, not imag bin 0
                    stx = sx_pool.tile([1, 2 * FP], BF16, tag="stx")
                    nc.gpsimd.memset(stx[:], 0)
                    nc.vector.tensor_copy(stx[:, 0:2 * FP:2], ps_i[0:1, :])
                    nc.gpsimd.memset(st[0:1, 1:2 * FP:2], 0)
                    nc.sync.dma_start(out[b, 512:513, :, :], stx[:, 0:2 * F])
                nc.sync.dma_start(
                    out[b, 128 * r:128 * r + 128, :, :], st[:, 0:2 * F]
                )

    nc.compile()
    return nc


def _host_prep_basis(basis: np.ndarray):
    # rows: [real 0..511 | real512, imag 1..127 | imag 128..255 | ... ]
    # (imag bin k = basis row 513 + k; imag 0 and imag 512 are zero rows)
    order = np.concatenate([np.arange(0, 512), [512], np.arange(514, 1025)])
    bt = basis[order].T.astype(np.float32)          # [1024 n, 1024 rows]
    # col layout: [r, c, re|im, 128] -> bt_sb[p, r*2048 + c*256 + side*128 + m]
    # where n = 128 c + p, re row = 128 r + m, im row = 512 + 128 r + m
    b4 = bt.reshape(8, 128, 2, 4, 128)              # [c, p, side, r, m]
    bt_sb = np.ascontiguousarray(b4.transpose(1, 3, 0, 2, 4).reshape(128, 8192))
    return bt_sb


def _get_exec():
    """Build (once) and return a cached executor fn(in_maps) -> full output."""
    if "exec" in _STATE:
        return _STATE["exec"]

    from concourse import bass2jax

    nc = _build_nc()

    def run(in_maps):
        res = bass2jax.run_bass_via_pjrt(nc, in_maps, n_cores=NCORES)
        return np.concatenate(
            [np.asarray(r["out"]).astype(np.float32) for r in res], axis=0
        )

    _STATE["exec"] = run
    return run


def _prep_inputs(x: np.ndarray, basis: np.ndarray):
    import ml_dtypes

    xp_all = np.zeros((B, L), ml_dtypes.bfloat16)
    xp_all[:, PAD:PAD + T] = np.asarray(x, np.float32).astype(ml_dtypes.bfloat16)
    bt_sb = _host_prep_basis(np.asarray(basis, np.float32)).astype(ml_dtypes.bfloat16)
    in_maps = [
        {
            "xp": xp_all[BPC * c:BPC * (c + 1)],
            "bt": bt_sb,
        }
        for c in range(NCORES)
    ]
    return in_maps


def kernel(x: np.ndarray, basis: np.ndarray) -> np.ndarray:
    run = _get_exec()
    in_maps = _prep_inputs(x, basis)
    return run(in_maps)                            # [32, 513, 501, 2]


# revision 20
# speedup vs baseline: 1.9911x; 1.2459x over previous
"""ConvSTFT kernel for Trainium2 (Bass/Tile), data-parallel over batch on 8 cores.

Math: out[b, k, f, i] = sum_n xp[b, 320 f + n] * basis[i*513 + k, n]
where xp = x padded with 512 zeros on both sides, f in [0, 501), n in [0, 1024).

Layout: Xs[p, j] = xp[p + 64 j] via the DMA XBAR transpose; contraction chunk
c of frame f is the stride-5 view Xs[:, 2c + 5f].

The 1024-point windowed DFT is factored with a 2-level decimation-in-
frequency split (twiddles folded into the second-stage matrices), computed on
the vector engines as chunkwise butterflies:

    y  = w * frame          (window, per-partition scalars)
    s  = y[:512] + y[512:]      d = y[:512] - y[512:]
    ss = s[:256] + s[256:]      sd = s[:256] - s[256:]
    odd bins 2m+1 = DFT-rows(d),  bins 4t+2 = DFT-rows(sd),
    bins 4t (incl. 512) = DFT-rows(ss)

which cuts the tensor-engine work from 64 to 24 matmuls (502 cycles each)
per batch. All data is bf16 (fp32 PSUM accumulation); output is written bf16
and widened on the host — the 2e-2 error budget is ~3x above what this
costs. Bin 512 (re) rides in the zero imag-bin-0 column of the "4t" group
and is routed out by a tiny fixup.
"""

import numpy as np
from contextlib import ExitStack

import concourse.bass as bass
import concourse.tile as tile
from concourse import bacc, mybir

# problem constants (hardcoded per harness contract)
B, T = 32, 160000
NCORES = 8
BPC = B // NCORES          # batches per core
HOP, NFFT = 320, 1024
BINS, F = 513, 501         # freq bins, frames
FP = 502                   # frames padded to even
PAD = NFFT // 2            # 512
JC = 2560                  # Xs columns (>= 2*7 + 5*(FP-1) + 1, mult of 16)
L = 127 + 64 * (JC - 1) + 1  # 163904 padded xp length
BF16 = mybir.dt.bfloat16
FTW = 3072 + 16            # 24 lhs tiles + 8 fp32 window vectors (bitcast)

_STATE: dict = {}


def _build_nc():
    nc = bacc.Bacc(
        "TRN2", target_bir_lowering=False, debug=False, num_devices=NCORES
    )
    f32 = mybir.dt.float32
    add, sub, mult = (
        mybir.AluOpType.add, mybir.AluOpType.subtract, mybir.AluOpType.mult
    )
    xp = nc.dram_tensor("xp", [BPC, L], BF16, kind="ExternalInput").ap()
    ft = nc.dram_tensor("ft", [128, FTW], BF16, kind="ExternalInput").ap()
    out = nc.dram_tensor("out", [BPC, BINS, F, 2], BF16, kind="ExternalOutput").ap()

    with tile.TileContext(nc) as tc, ExitStack() as ctx:
        const_pool = ctx.enter_context(tc.tile_pool(name="const", bufs=1))
        xs_pool = ctx.enter_context(tc.tile_pool(name="xs", bufs=2))
        bf_pool = ctx.enter_context(tc.tile_pool(name="bf", bufs=2))
        st_pool = ctx.enter_context(tc.tile_pool(name="st", bufs=3))
        sx_pool = ctx.enter_context(tc.tile_pool(name="sx", bufs=2))
        acc_pool = ctx.enter_context(tc.tile_pool(name="acc", bufs=2, space="PSUM"))
        wu_pool = ctx.enter_context(tc.tile_pool(name="wu", bufs=1))
        wup_pool = ctx.enter_context(tc.tile_pool(name="wup", bufs=1, space="PSUM"))

        # PE p-state warmup: keep the tensor engine's busy-streak alive
        # through the startup DMA window so real matmuls run at full clock
        dummy = wu_pool.tile([128, 512], BF16, tag="dummy")
        nc.vector.memset(dummy[:], 0)
        pw = wup_pool.tile([128, 512], f32, tag="pw")
        for _ in range(33):
            nc.tensor.matmul(pw[:], dummy[:, 0:128], dummy[:], start=True, stop=True)

        # startup DMA chain (serialized; ~2us/hop): odd-group lhs + windows
        # first, then batch-0 xbar, then the rest of the lhs matrices
        ft_sb = const_pool.tile([128, FTW], BF16, tag="ft")
        nc.scalar.dma_start(ft_sb[:, 3072:FTW], ft[:, 3072:FTW])
        nc.scalar.dma_start(ft_sb[:, 0:1024], ft[:, 0:1024])

        xs0 = xs_pool.tile([128, JC], BF16, tag="xs")
        nc.sync.dma_start_transpose(
            xs0[:], bass.AP(xp.tensor, 0, [[64, JC], [1, 128]])
        )

        wt_last = nc.scalar.dma_start(ft_sb[:, 1024:3072], ft[:, 1024:3072])

        f32w = lambda o: ft_sb[:, o:o + 2].bitcast(f32)
        wl = [f32w(3072 + 2 * c) for c in range(4)]
        wh = [f32w(3080 + 2 * c) for c in range(4)]

        for b in range(BPC):
            if b == 0:
                xs = xs0
            else:
                xs = xs_pool.tile([128, JC], BF16, tag="xs")
                src = bass.AP(xp.tensor, b * L, [[64, JC], [1, 128]])
                xbi = nc.sync.dma_start_transpose(xs[:], src)
                if b == 1:
                    # keep this prefetch behind the last lhs load in the
                    # serialized DMA chain: it gates batch-0 compute
                    import bass_rust as _br
                    xbi.ins.add_dependency(
                        wt_last.ins.name, _br.DependencyInfo(sync=True, no_sync=False)
                    )

            def view(c):
                return xs[:, 2 * c: 2 * c + 5 * FP: 5]

            # butterflies: t = wh*y_hi (Act); s/d = wl*y_lo +- t (DVE STT);
            # ss/sd chunk add/sub on Pool (STT is not a Pool-legal opcode)
            tv, sv, dv = [], [], []
            for c in range(4):
                t_c = bf_pool.tile([128, FP], BF16, tag=f"t{c}")
                nc.scalar.mul(t_c[:], view(c + 4), wh[c])
                s_c = bf_pool.tile([128, FP], BF16, tag=f"s{c}")
                nc.vector.scalar_tensor_tensor(
                    s_c[:], view(c), wl[c], t_c[:], mult, add
                )
                d_c = bf_pool.tile([128, FP], BF16, tag=f"d{c}")
                nc.vector.scalar_tensor_tensor(
                    d_c[:], view(c), wl[c], t_c[:], mult, sub
                )
                tv.append(t_c); sv.append(s_c); dv.append(d_c)
            ssv, sdv = [], []
            for ch in range(2):
                ss_c = bf_pool.tile([128, FP], BF16, tag=f"ss{ch}")
                nc.gpsimd.tensor_tensor(ss_c[:], sv[ch][:], sv[ch + 2][:], add)
                sd_c = bf_pool.tile([128, FP], BF16, tag=f"sd{ch}")
                nc.gpsimd.tensor_tensor(sd_c[:], sv[ch][:], sv[ch + 2][:], sub)
                ssv.append(ss_c); sdv.append(sd_c)

            # groups: (lhs col base, rhs chunk list, out row base, out row step)
            groups = [
                (0, dv, 1, 2),        # odd bins 1,3,..,255*2+1: m=0..127
                (1024, dv, 257, 2),   # odd bins 257,..,511: m=128..255
                (2048, sdv, 2, 4),    # bins 4t+2
                (2560, ssv, 0, 4),    # bins 4t (+ bin 512 fixup in im col 0)
            ]
            for gi, (base, rhs_l, k0, kstep) in enumerate(groups):
                ps_r = acc_pool.tile([128, FP], f32, tag="accr")
                ps_i = acc_pool.tile([128, FP], f32, tag="acci")
                nch = len(rhs_l)
                for ci in range(nch):
                    lr = ft_sb[:, base + ci * 256: base + ci * 256 + 128]
                    li = ft_sb[:, base + ci * 256 + 128: base + ci * 256 + 256]
                    nc.tensor.matmul(
                        ps_r[:], lr, rhs_l[ci][:], start=(ci == 0), stop=(ci == nch - 1)
                    )
                    nc.tensor.matmul(
                        ps_i[:], li, rhs_l[ci][:], start=(ci == 0), stop=(ci == nch - 1)
                    )
                st = st_pool.tile([128, 2 * FP], BF16, tag="st")
                nc.vector.tensor_copy(st[:, 0:2 * FP:2], ps_r[:])
                nc.scalar.copy(st[:, 1:2 * FP:2], ps_i[:])
                if gi == 3:
                    # partition 0 of ps_i holds real bin 512, not imag bin 0
                    stx = sx_pool.tile([1, 2 * FP], BF16, tag="stx")
                    nc.gpsimd.memset(stx[:], 0)
                    nc.scalar.copy(stx[:, 0:2 * FP:2], ps_i[0:1, :])
                    nc.gpsimd.memset(st[0:1, 1:2 * FP:2], 0)
                    nc.sync.dma_start(out[b, 512:513, :, :], stx[:, 0:2 * F])
                dst = bass.AP(
                    out.tensor, (b * BINS + k0) * F * 2,
                    [[kstep * F * 2, 128], [1, F * 2]],
                )
                nc.sync.dma_start(dst, st[:, 0:2 * F])

    nc.compile()
    return nc


def _host_prep_basis(basis: np.ndarray):
    """Build the 2-level DIF lhs matrices + window columns, [128, FTW]."""
    w = np.asarray(basis, np.float64)[0]            # basis row 0 = window
    ftc = np.zeros((128, FTW), np.float64)
    p = np.arange(128)[:, None]
    j = np.arange(128)[None, :]

    def dft_block(qbase, kof, kstep):
        ang = 2.0 * np.pi * ((qbase + p) * (kof + kstep * j)) / NFFT
        return np.cos(ang), -np.sin(ang)

    for g in range(2):          # odd bins 2m+1, m = 128g + j, q = 128c + p
        for c in range(4):
            cosb, sinb = dft_block(128 * c, 2 * (128 * g) + 1, 2)
            ftc[:, g * 1024 + c * 256:g * 1024 + c * 256 + 128] = cosb
            ftc[:, g * 1024 + c * 256 + 128:g * 1024 + c * 256 + 256] = sinb
    for c in range(2):          # bins 4t+2, u = 128c + p
        cosb, sinb = dft_block(128 * c, 2, 4)
        ftc[:, 2048 + c * 256:2048 + c * 256 + 128] = cosb
        ftc[:, 2048 + c * 256 + 128:2048 + c * 256 + 256] = sinb
    for c in range(2):          # bins 4t, u = 128c + p; im col 0 -> bin512 re
        cosb, sinb = dft_block(128 * c, 0, 4)
        sinb = sinb.copy()
        sinb[:, 0] = np.cos(np.pi * (128 * c + np.arange(128)))   # (-1)^u
        ftc[:, 2560 + c * 256:2560 + c * 256 + 128] = cosb
        ftc[:, 2560 + c * 256 + 128:2560 + c * 256 + 256] = sinb
    return ftc, w


def _get_exec():
    """Build (once) and return a cached executor fn(in_maps) -> full output."""
    if "exec" in _STATE:
        return _STATE["exec"]

    from concourse import bass2jax

    nc = _build_nc()

    def run(in_maps):
        res = bass2jax.run_bass_via_pjrt(nc, in_maps, n_cores=NCORES)
        return np.concatenate(
            [np.asarray(r["out"]).astype(np.float32) for r in res], axis=0
        )

    _STATE["exec"] = run
    return run


def _prep_inputs(x: np.ndarray, basis: np.ndarray):
    import ml_dtypes

    xp_all = np.zeros((B, L), ml_dtypes.bfloat16)
    xp_all[:, PAD:PAD + T] = np.asarray(x, np.float32).astype(ml_dtypes.bfloat16)
    ftc, w = _host_prep_basis(basis)
    ft = ftc.astype(ml_dtypes.bfloat16)
    # window vectors as raw fp32 bits in the bf16 tensor (kernel bitcasts)
    w8 = np.stack(
        [w[128 * c:128 * c + 128] for c in range(4)]
        + [w[512 + 128 * c:512 + 128 * c + 128] for c in range(4)], axis=1
    ).astype(np.float32)                            # [128, 8]
    ft.view(np.uint16)[:, 3072:3088] = w8.view(np.uint16)
    in_maps = [
        {
            "xp": xp_all[BPC * c:BPC * (c + 1)],
            "ft": ft,
        }
        for c in range(NCORES)
    ]
    return in_maps


def kernel(x: np.ndarray, basis: np.ndarray) -> np.ndarray:
    run = _get_exec()
    in_maps = _prep_inputs(x, basis)
    return run(in_maps)                            # [32, 513, 501, 2]


# revision 36
# speedup vs baseline: 2.7013x; 1.3567x over previous
"""ConvSTFT kernel for Trainium2 (Bass/Tile), data-parallel over batch on 8 cores.

Math: out[b, k, f, i] = sum_n xp[b, 320 f + n] * basis[i*513 + k, n]
where xp = x padded with 512 zeros on both sides, f in [0, 501), n in [0, 1024).

Layout: Xs[p, j] = xp[p + 64 j] via the DMA XBAR transpose; contraction chunk
c of frame f is the stride-5 view Xs[:, 2c + 5f].

The 1024-point windowed DFT is factored with a 2-level decimation-in-
frequency split (twiddles folded into the second-stage matrices), computed on
the vector engines as chunkwise butterflies:

    y  = w * frame          (window, per-partition scalars)
    s  = y[:512] + y[512:]      d = y[:512] - y[512:]
    ss = s[:256] + s[256:]      sd = s[:256] - s[256:]
    odd bins 2m+1 = DFT-rows(d),  bins 4t+2 = DFT-rows(sd),
    bins 4t (incl. 512) = DFT-rows(ss)

which cuts the tensor-engine work from 64 to 24 matmuls (502 cycles each)
per batch. All data is bf16 (fp32 PSUM accumulation); output is written bf16
and widened on the host — the 2e-2 error budget is ~3x above what this
costs. Bin 512 (re) rides in the zero imag-bin-0 column of the "4t" group
and is routed out by a tiny fixup.
"""

import numpy as np
from contextlib import ExitStack

import concourse.bass as bass
import concourse.tile as tile
from concourse import bacc, mybir

# problem constants (hardcoded per harness contract)
B, T = 32, 160000
NCORES = 8
BPC = B // NCORES          # batches per core
HOP, NFFT = 320, 1024
BINS, F = 513, 501         # freq bins, frames
FP = 502                   # frames padded to even
PAD = NFFT // 2            # 512
JC = 2560                  # Xs columns (>= 2*7 + 5*(FP-1) + 1, mult of 16)
L = 127 + 64 * (JC - 1) + 1  # 163904 padded xp length
BF16 = mybir.dt.bfloat16
FTW = 3072 + 16            # 24 lhs tiles + 8 fp32 window vectors (bitcast)

_STATE: dict = {}


def _build_nc():
    nc = bacc.Bacc(
        "TRN2", target_bir_lowering=False, debug=False, num_devices=NCORES
    )
    f32 = mybir.dt.float32
    add, sub, mult = (
        mybir.AluOpType.add, mybir.AluOpType.subtract, mybir.AluOpType.mult
    )
    xst = nc.dram_tensor("xst", [BPC, 128, JC], BF16, kind="ExternalInput").ap()
    ft = nc.dram_tensor("ft", [128, FTW], BF16, kind="ExternalInput").ap()
    out = nc.dram_tensor("out", [BPC, BINS, F, 2], BF16, kind="ExternalOutput").ap()

    with tile.TileContext(nc) as tc, ExitStack() as ctx:
        const_pool = ctx.enter_context(tc.tile_pool(name="const", bufs=1))
        xs_pool = ctx.enter_context(tc.tile_pool(name="xs", bufs=4))
        bf_pool = ctx.enter_context(tc.tile_pool(name="bf", bufs=4))
        st_pool = ctx.enter_context(tc.tile_pool(name="st", bufs=4))
        sx_pool = ctx.enter_context(tc.tile_pool(name="sx", bufs=2))
        acc_pool = ctx.enter_context(tc.tile_pool(name="acc", bufs=3, space="PSUM"))
        wu_pool = ctx.enter_context(tc.tile_pool(name="wu", bufs=1))
        wup_pool = ctx.enter_context(tc.tile_pool(name="wup", bufs=1, space="PSUM"))

        # PE p-state warmup: keep the tensor engine's busy-streak alive
        # through the startup DMA window so real matmuls run at full clock
        dummy = wu_pool.tile([128, 512], BF16, tag="dummy")
        nc.vector.memset(dummy[:], 0)
        pw = wup_pool.tile([128, 512], f32, tag="pw")
        for _ in range(20):
            nc.tensor.matmul(pw[:], dummy[:, 0:128], dummy[:], start=True, stop=True)

        # startup DMA chain (serialized, ~1.5-2us/hop): window scalars, then
        # the batch-0 xbar (gates the butterflies), then the lhs matrices in
        # consumption order
        ft_sb = const_pool.tile([128, FTW], BF16, tag="ft")
        nc.scalar.dma_start(ft_sb[:, 3072:FTW], ft[:, 3072:FTW])

        import bass_rust as _br

        def chain(a, bj):
            bj.ins.add_dependency(
                a.ins.name, _br.DependencyInfo(sync=True, no_sync=False)
            )

        # input loads: host has already laid x out transposed (xst[b, p, j]
        # = padded x[b, p + 64 j]), so these are plain contiguous DMAs
        xs_t, xs_i = [], []
        for b in range(BPC):
            xs = xs_pool.tile([128, JC], BF16, tag="xs")
            xs_t.append(xs)
            xs_i.append(nc.sync.dma_start(xs[:], xst[b]))

        ft1 = nc.scalar.dma_start(ft_sb[:, 0:2048], ft[:, 0:2048])
        ft2 = nc.scalar.dma_start(ft_sb[:, 2048:3072], ft[:, 2048:3072])
        # shape the serialized DMA chain: xs0, ft(odd), xs1, ft(rest), xs2/3
        chain(xs_i[0], ft1)
        chain(ft1, xs_i[1])
        chain(xs_i[1], ft2)
        chain(ft2, xs_i[2])

        f32w = lambda o: ft_sb[:, o:o + 2].bitcast(f32)
        wl = [f32w(3072 + 2 * c) for c in range(4)]
        wh = [f32w(3080 + 2 * c) for c in range(4)]

        # stx buffers are zeroed once at startup; per-batch writes touch only
        # the even slots, the odd slots stay zero (imag of bin 512 is 0)
        stx_bufs = []
        for _ in range(2):
            sx = sx_pool.tile([1, 2 * FP], BF16, tag="stx")
            nc.gpsimd.memset(sx[:], 0)
            stx_bufs.append(sx)

        for b in range(BPC):
            xs = xs_t[b]

            def view(c):
                return xs[:, 2 * c: 2 * c + 5 * FP: 5]

            # butterflies, engine-balanced (STT has no DVE fast path, so use
            # tensor_scalar + tensor_tensor which run at 2x):
            #   u = wl*y_lo, t = wh*y_hi   (TS: DVE x7, Act x1)
            #   d = u - t, s = u + t       (TT: d + s23 on DVE, s01 on Pool)
            #   ss = s0+s2 (DVE), sd = s0-s2 (Pool)
            uv, tv = [], []
            for c in range(4):
                u_c = bf_pool.tile([128, FP], BF16, tag=f"u{c}")
                nc.vector.tensor_scalar_mul(u_c[:], view(c), wl[c])
                t_c = bf_pool.tile([128, FP], BF16, tag=f"t{c}")
                nc.vector.tensor_scalar_mul(t_c[:], view(c + 4), wh[c])
                uv.append(u_c); tv.append(t_c)
            dv, sv = [], [None] * 4
            for c in range(4):
                d_c = bf_pool.tile([128, FP], BF16, tag=f"d{c}")
                nc.vector.tensor_tensor(d_c[:], uv[c][:], tv[c][:], sub)
                dv.append(d_c)
            for c in range(4):
                s_c = bf_pool.tile([128, FP], BF16, tag=f"s{c}")
                eng = nc.gpsimd if c < 2 else nc.vector
                eng.tensor_tensor(s_c[:], uv[c][:], tv[c][:], add)
                sv[c] = s_c
            ssv, sdv = [], []
            for ch in range(2):
                ss_c = bf_pool.tile([128, FP], BF16, tag=f"ss{ch}")
                nc.vector.tensor_tensor(ss_c[:], sv[ch][:], sv[ch + 2][:], add)
                ssv.append(ss_c)
            for ch in range(2):
                sd_c = bf_pool.tile([128, FP], BF16, tag=f"sd{ch}")
                nc.gpsimd.tensor_tensor(sd_c[:], sv[ch][:], sv[ch + 2][:], sub)
                sdv.append(sd_c)

            # groups ordered by rhs readiness: d (DVE, early) -> ss (DVE) ->
            # sd (Pool, latest); (lhs base, rhs chunks, out row base, step).
            # The last batch ends on an odd group (no fixup ops in the tail).
            groups = [
                (0, dv, 1, 2, False),        # odd bins 1,3,..,255: m=0..127
                (1024, dv, 257, 2, False),   # odd bins 257,..,511: m=128..255
                (2560, ssv, 0, 4, True),     # bins 4t (+ bin 512 fixup)
                (2048, sdv, 2, 4, False),    # bins 4t+2
            ]
            if b == BPC - 1:
                groups = [groups[2], groups[3], groups[0], groups[1]]
            for base, rhs_l, k0, kstep, is_ee in groups:
                # one 2-bank PSUM tile: re half at col 0, im half at col 512,
                # so a single multi-dim copy interleaves both into st
                ps = acc_pool.tile([128, 1024], f32, tag="acc")
                nch = len(rhs_l)
                for ci in range(nch):
                    lr = ft_sb[:, base + ci * 256: base + ci * 256 + 128]
                    li = ft_sb[:, base + ci * 256 + 128: base + ci * 256 + 256]
                    nc.tensor.matmul(
                        ps[:, 0:FP], lr, rhs_l[ci][:],
                        start=(ci == 0), stop=(ci == nch - 1)
                    )
                    nc.tensor.matmul(
                        ps[:, 512:512 + FP], li, rhs_l[ci][:],
                        start=(ci == 0), stop=(ci == nch - 1)
                    )
                if is_ee:
                    # partition 0 of the im half holds real bin 512, not the
                    # (identically zero) imag bin 0: route it out via stx,
                    # then zero the odd slots of row 0 (imag bin 0)
                    st = st_pool.tile([128, 2 * FP], BF16, tag="st")
                    st3 = st[:].rearrange("p (f i) -> p f i", i=2)
                    ps3 = ps[:].rearrange("p (i f) -> p f i", i=2)[:, 0:FP, :]
                    nc.scalar.copy(st3, ps3)
                    nc.vector.memset(st[0:1, 1:2 * FP:2], 0)
                    stx = stx_bufs[b % 2]
                    nc.scalar.copy(stx[:, 0:2 * FP:2], ps[0:1, 512:512 + FP])
                    nc.sync.dma_start(out[b, 512:513, :, :], stx[:, 0:2 * F])
                else:
                    st = st_pool.tile([128, 2 * FP], BF16, tag="st")
                    st3 = st[:].rearrange("p (f i) -> p f i", i=2)
                    ps3 = ps[:].rearrange("p (i f) -> p f i", i=2)[:, 0:FP, :]
                    # final batch: split drains across DVE+Act so the tail
                    # chain is half as long (DVE is idle by then)
                    if b == BPC - 1 and k0 in (1, 2):
                        nc.vector.tensor_copy(st3, ps3)
                    else:
                        nc.scalar.copy(st3, ps3)
                dst = bass.AP(
                    out.tensor, (b * BINS + k0) * F * 2,
                    [[kstep * F * 2, 128], [1, F * 2]],
                )
                nc.sync.dma_start(dst, st[:, 0:2 * F])

    nc.compile()
    return nc


def _host_prep_basis(basis: np.ndarray):
    """Build the 2-level DIF lhs matrices + window columns, [128, FTW]."""
    w = np.asarray(basis, np.float64)[0]            # basis row 0 = window
    ftc = np.zeros((128, FTW), np.float64)
    p = np.arange(128)[:, None]
    j = np.arange(128)[None, :]

    def dft_block(qbase, kof, kstep):
        ang = 2.0 * np.pi * ((qbase + p) * (kof + kstep * j)) / NFFT
        return np.cos(ang), -np.sin(ang)

    for g in range(2):          # odd bins 2m+1, m = 128g + j, q = 128c + p
        for c in range(4):
            cosb, sinb = dft_block(128 * c, 2 * (128 * g) + 1, 2)
            ftc[:, g * 1024 + c * 256:g * 1024 + c * 256 + 128] = cosb
            ftc[:, g * 1024 + c * 256 + 128:g * 1024 + c * 256 + 256] = sinb
    for c in range(2):          # bins 4t+2, u = 128c + p
        cosb, sinb = dft_block(128 * c, 2, 4)
        ftc[:, 2048 + c * 256:2048 + c * 256 + 128] = cosb
        ftc[:, 2048 + c * 256 + 128:2048 + c * 256 + 256] = sinb
    for c in range(2):          # bins 4t, u = 128c + p; im col 0 -> bin512 re
        cosb, sinb = dft_block(128 * c, 0, 4)
        sinb = sinb.copy()
        sinb[:, 0] = np.cos(np.pi * (128 * c + np.arange(128)))   # (-1)^u
        ftc[:, 2560 + c * 256:2560 + c * 256 + 128] = cosb
        ftc[:, 2560 + c * 256 + 128:2560 + c * 256 + 256] = sinb
    return ftc, w


def _get_exec():
    """Build (once) and return a cached executor fn(in_maps) -> full output."""
    if "exec" in _STATE:
        return _STATE["exec"]

    from concourse import bass2jax

    nc = _build_nc()

    def run(in_maps):
        res = bass2jax.run_bass_via_pjrt(nc, in_maps, n_cores=NCORES)
        return np.concatenate(
            [np.asarray(r["out"]).astype(np.float32) for r in res], axis=0
        )

    _STATE["exec"] = run
    return run


def _prep_inputs(x: np.ndarray, basis: np.ndarray):
    import ml_dtypes

    xp_all = np.zeros((B, L), ml_dtypes.bfloat16)
    xp_all[:, PAD:PAD + T] = np.asarray(x, np.float32).astype(ml_dtypes.bfloat16)
    # transposed overlap layout: xst[b, p, j] = xp[b, p + 64 j]
    sb = xp_all.strides
    xst = np.ascontiguousarray(np.lib.stride_tricks.as_strided(
        xp_all, (B, 128, JC), (sb[0], sb[1], 64 * sb[1])))
    ftc, w = _host_prep_basis(basis)
    ft = ftc.astype(ml_dtypes.bfloat16)
    # window vectors as raw fp32 bits in the bf16 tensor (kernel bitcasts)
    w8 = np.stack(
        [w[128 * c:128 * c + 128] for c in range(4)]
        + [w[512 + 128 * c:512 + 128 * c + 128] for c in range(4)], axis=1
    ).astype(np.float32)                            # [128, 8]
    ft.view(np.uint16)[:, 3072:3088] = w8.view(np.uint16)
    in_maps = [
        {
            "xst": xst[BPC * c:BPC * (c + 1)],
            "ft": ft,
        }
        for c in range(NCORES)
    ]
    return in_maps


def kernel(x: np.ndarray, basis: np.ndarray) -> np.ndarray:
    run = _get_exec()
    in_maps = _prep_inputs(x, basis)
    return run(in_maps)                            # [32, 513, 501, 2]


# revision 37
# speedup vs baseline: 2.8775x; 1.0652x over previous
"""ConvSTFT kernel for Trainium2 (Bass/Tile), data-parallel over batch on 8 cores.

Math: out[b, k, f, i] = sum_n xp[b, 320 f + n] * basis[i*513 + k, n]
where xp = x padded with 512 zeros on both sides, f in [0, 501), n in [0, 1024).

Layout: Xs[p, j] = xp[p + 64 j] via the DMA XBAR transpose; contraction chunk
c of frame f is the stride-5 view Xs[:, 2c + 5f].

The 1024-point windowed DFT is factored with a 2-level decimation-in-
frequency split (twiddles folded into the second-stage matrices), computed on
the vector engines as chunkwise butterflies:

    y  = w * frame          (window, per-partition scalars)
    s  = y[:512] + y[512:]      d = y[:512] - y[512:]
    ss = s[:256] + s[256:]      sd = s[:256] - s[256:]
    odd bins 2m+1 = DFT-rows(d),  bins 4t+2 = DFT-rows(sd),
    bins 4t (incl. 512) = DFT-rows(ss)

which cuts the tensor-engine work from 64 to 24 matmuls (502 cycles each)
per batch. All data is bf16 (fp32 PSUM accumulation); output is written bf16
and widened on the host — the 2e-2 error budget is ~3x above what this
costs. Bin 512 (re) rides in the zero imag-bin-0 column of the "4t" group
and is routed out by a tiny fixup.
"""

import numpy as np
from contextlib import ExitStack

import concourse.bass as bass
import concourse.tile as tile
from concourse import bacc, mybir

# problem constants (hardcoded per harness contract)
B, T = 32, 160000
NCORES = 8
BPC = B // NCORES          # batches per core
HOP, NFFT = 320, 1024
BINS, F = 513, 501         # freq bins, frames
FP = 502                   # frames padded to even
PAD = NFFT // 2            # 512
JC = 2560                  # Xs columns (>= 2*7 + 5*(FP-1) + 1, mult of 16)
L = 127 + 64 * (JC - 1) + 1  # 163904 padded xp length
BF16 = mybir.dt.bfloat16
FTW = 3072 + 16            # 24 lhs tiles + 8 fp32 window vectors (bitcast)

_STATE: dict = {}


def _build_nc():
    nc = bacc.Bacc(
        "TRN2", target_bir_lowering=False, debug=False, num_devices=NCORES
    )
    f32 = mybir.dt.float32
    add, sub, mult = (
        mybir.AluOpType.add, mybir.AluOpType.subtract, mybir.AluOpType.mult
    )
    xst = nc.dram_tensor("xst", [BPC, 128, JC], BF16, kind="ExternalInput").ap()
    ft = nc.dram_tensor("ft", [128, FTW], BF16, kind="ExternalInput").ap()
    out = nc.dram_tensor("out", [BPC, BINS, F, 2], BF16, kind="ExternalOutput").ap()

    with tile.TileContext(nc) as tc, ExitStack() as ctx:
        const_pool = ctx.enter_context(tc.tile_pool(name="const", bufs=1))
        xs_pool = ctx.enter_context(tc.tile_pool(name="xs", bufs=4))
        bf_pool = ctx.enter_context(tc.tile_pool(name="bf", bufs=4))
        st_pool = ctx.enter_context(tc.tile_pool(name="st", bufs=4))
        sx_pool = ctx.enter_context(tc.tile_pool(name="sx", bufs=2))
        acc_pool = ctx.enter_context(tc.tile_pool(name="acc", bufs=3, space="PSUM"))
        wu_pool = ctx.enter_context(tc.tile_pool(name="wu", bufs=1))
        wup_pool = ctx.enter_context(tc.tile_pool(name="wup", bufs=1, space="PSUM"))

        # PE p-state warmup: keep the tensor engine's busy-streak alive
        # through the startup DMA window so real matmuls run at full clock
        dummy = wu_pool.tile([128, 512], BF16, tag="dummy")
        nc.vector.memset(dummy[:], 0)
        pw = wup_pool.tile([128, 512], f32, tag="pw")
        for _ in range(17):
            nc.tensor.matmul(pw[:], dummy[:, 0:128], dummy[:], start=True, stop=True)

        # startup DMA chain (serialized, ~1.5-2us/hop): window scalars, then
        # the batch-0 xbar (gates the butterflies), then the lhs matrices in
        # consumption order
        ft_sb = const_pool.tile([128, FTW], BF16, tag="ft")
        nc.scalar.dma_start(ft_sb[:, 3072:FTW], ft[:, 3072:FTW])

        import bass_rust as _br

        def chain(a, bj):
            bj.ins.add_dependency(
                a.ins.name, _br.DependencyInfo(sync=True, no_sync=False)
            )

        # input loads: host has already laid x out transposed (xst[b, p, j]
        # = padded x[b, p + 64 j]), so these are plain contiguous DMAs
        xs_t, xs_i = [], []
        for b in range(BPC):
            xs = xs_pool.tile([128, JC], BF16, tag="xs")
            xs_t.append(xs)
            xs_i.append(nc.sync.dma_start(xs[:], xst[b]))

        ft1 = nc.scalar.dma_start(ft_sb[:, 0:3072], ft[:, 0:3072])
        # shape the serialized DMA chain: xs0, ft, xs1, xs2, xs3
        chain(xs_i[0], ft1)
        chain(ft1, xs_i[1])
        chain(xs_i[1], xs_i[2])
        chain(xs_i[2], xs_i[3])

        f32w = lambda o: ft_sb[:, o:o + 2].bitcast(f32)
        wl = [f32w(3072 + 2 * c) for c in range(4)]
        wh = [f32w(3080 + 2 * c) for c in range(4)]

        # stx buffers are zeroed once at startup; per-batch writes touch only
        # the even slots, the odd slots stay zero (imag of bin 512 is 0)
        stx_bufs = []
        for _ in range(2):
            sx = sx_pool.tile([1, 2 * FP], BF16, tag="stx")
            nc.gpsimd.memset(sx[:], 0)
            stx_bufs.append(sx)

        for b in range(BPC):
            xs = xs_t[b]

            def view(c):
                return xs[:, 2 * c: 2 * c + 5 * FP: 5]

            # butterflies, engine-balanced (STT has no DVE fast path, so use
            # tensor_scalar + tensor_tensor which run at 2x):
            #   u = wl*y_lo, t = wh*y_hi   (TS: DVE x7, Act x1)
            #   d = u - t, s = u + t       (TT: d + s23 on DVE, s01 on Pool)
            #   ss = s0+s2 (DVE), sd = s0-s2 (Pool)
            uv, tv = [], []
            for c in range(4):
                u_c = bf_pool.tile([128, FP], BF16, tag=f"u{c}")
                nc.vector.tensor_scalar_mul(u_c[:], view(c), wl[c])
                t_c = bf_pool.tile([128, FP], BF16, tag=f"t{c}")
                nc.vector.tensor_scalar_mul(t_c[:], view(c + 4), wh[c])
                uv.append(u_c); tv.append(t_c)
            dv, sv = [], [None] * 4
            for c in range(4):
                d_c = bf_pool.tile([128, FP], BF16, tag=f"d{c}")
                nc.vector.tensor_tensor(d_c[:], uv[c][:], tv[c][:], sub)
                dv.append(d_c)
            for c in range(4):
                s_c = bf_pool.tile([128, FP], BF16, tag=f"s{c}")
                eng = nc.gpsimd if c < 2 else nc.vector
                eng.tensor_tensor(s_c[:], uv[c][:], tv[c][:], add)
                sv[c] = s_c
            ssv, sdv = [], []
            for ch in range(2):
                ss_c = bf_pool.tile([128, FP], BF16, tag=f"ss{ch}")
                nc.vector.tensor_tensor(ss_c[:], sv[ch][:], sv[ch + 2][:], add)
                ssv.append(ss_c)
            for ch in range(2):
                sd_c = bf_pool.tile([128, FP], BF16, tag=f"sd{ch}")
                nc.gpsimd.tensor_tensor(sd_c[:], sv[ch][:], sv[ch + 2][:], sub)
                sdv.append(sd_c)

            # groups ordered by rhs readiness: d (DVE, early) -> ss (DVE) ->
            # sd (Pool, latest); (lhs base, rhs chunks, out row base, step).
            # The last batch ends on an odd group (no fixup ops in the tail).
            groups = [
                (0, dv, 1, 2, False),        # odd bins 1,3,..,255: m=0..127
                (1024, dv, 257, 2, False),   # odd bins 257,..,511: m=128..255
                (2560, ssv, 0, 4, True),     # bins 4t (+ bin 512 fixup)
                (2048, sdv, 2, 4, False),    # bins 4t+2
            ]
            if b == BPC - 1:
                groups = [groups[2], groups[3], groups[0], groups[1]]
            for base, rhs_l, k0, kstep, is_ee in groups:
                # one 2-bank PSUM tile: re half at col 0, im half at col 512,
                # so a single multi-dim copy interleaves both into st
                ps = acc_pool.tile([128, 1024], f32, tag="acc")
                nch = len(rhs_l)
                for ci in range(nch):
                    lr = ft_sb[:, base + ci * 256: base + ci * 256 + 128]
                    li = ft_sb[:, base + ci * 256 + 128: base + ci * 256 + 256]
                    nc.tensor.matmul(
                        ps[:, 0:FP], lr, rhs_l[ci][:],
                        start=(ci == 0), stop=(ci == nch - 1)
                    )
                    nc.tensor.matmul(
                        ps[:, 512:512 + FP], li, rhs_l[ci][:],
                        start=(ci == 0), stop=(ci == nch - 1)
                    )
                if is_ee:
                    # partition 0 of the im half holds real bin 512, not the
                    # (identically zero) imag bin 0: route it out via stx,
                    # then zero the odd slots of row 0 (imag bin 0)
                    st = st_pool.tile([128, 2 * FP], BF16, tag="st")
                    st3 = st[:].rearrange("p (f i) -> p f i", i=2)
                    ps3 = ps[:].rearrange("p (i f) -> p f i", i=2)[:, 0:FP, :]
                    nc.scalar.copy(st3, ps3)
                    nc.scalar.mul(st[0:1, 1:2 * FP:2], st[0:1, 1:2 * FP:2], 0.0)
                    stx = stx_bufs[b % 2]
                    nc.scalar.copy(stx[:, 0:2 * FP:2], ps[0:1, 512:512 + FP])
                    nc.sync.dma_start(out[b, 512:513, :, :], stx[:, 0:2 * F])
                else:
                    st = st_pool.tile([128, 2 * FP], BF16, tag="st")
                    st3 = st[:].rearrange("p (f i) -> p f i", i=2)
                    ps3 = ps[:].rearrange("p (i f) -> p f i", i=2)[:, 0:FP, :]
                    if b == BPC - 1 and k0 == 257:
                        # very last group: halve the drain across DVE+Act and
                        # DMA each half as it lands to shorten the tail
                        HF = FP // 2
                        nc.vector.tensor_copy(st3[:, 0:HF, :], ps3[:, 0:HF, :])
                        nc.scalar.copy(st3[:, HF:FP, :], ps3[:, HF:FP, :])
                        dst1 = bass.AP(
                            out.tensor, (b * BINS + k0) * F * 2,
                            [[kstep * F * 2, 128], [1, 2 * HF]],
                        )
                        dst2 = bass.AP(
                            out.tensor, (b * BINS + k0) * F * 2 + 2 * HF,
                            [[kstep * F * 2, 128], [1, 2 * F - 2 * HF]],
                        )
                        nc.sync.dma_start(dst1, st[:, 0:2 * HF])
                        nc.sync.dma_start(dst2, st[:, 2 * HF:2 * F])
                        continue
                    # final batch: split drains across DVE+Act so the tail
                    # chain is half as long (DVE is idle by then)
                    if b == BPC - 1 and k0 in (1, 2):
                        nc.vector.tensor_copy(st3, ps3)
                    else:
                        nc.scalar.copy(st3, ps3)
                dst = bass.AP(
                    out.tensor, (b * BINS + k0) * F * 2,
                    [[kstep * F * 2, 128], [1, F * 2]],
                )
                nc.sync.dma_start(dst, st[:, 0:2 * F])

    nc.compile()
    return nc


def _host_prep_basis(basis: np.ndarray):
    """Build the 2-level DIF lhs matrices + window columns, [128, FTW]."""
    w = np.asarray(basis, np.float64)[0]            # basis row 0 = window
    ftc = np.zeros((128, FTW), np.float64)
    p = np.arange(128)[:, None]
    j = np.arange(128)[None, :]

    def dft_block(qbase, kof, kstep):
        ang = 2.0 * np.pi * ((qbase + p) * (kof + kstep * j)) / NFFT
        return np.cos(ang), -np.sin(ang)

    for g in range(2):          # odd bins 2m+1, m = 128g + j, q = 128c + p
        for c in range(4):
            cosb, sinb = dft_block(128 * c, 2 * (128 * g) + 1, 2)
            ftc[:, g * 1024 + c * 256:g * 1024 + c * 256 + 128] = cosb
            ftc[:, g * 1024 + c * 256 + 128:g * 1024 + c * 256 + 256] = sinb
    for c in range(2):          # bins 4t+2, u = 128c + p
        cosb, sinb = dft_block(128 * c, 2, 4)
        ftc[:, 2048 + c * 256:2048 + c * 256 + 128] = cosb
        ftc[:, 2048 + c * 256 + 128:2048 + c * 256 + 256] = sinb
    for c in range(2):          # bins 4t, u = 128c + p; im col 0 -> bin512 re
        cosb, sinb = dft_block(128 * c, 0, 4)
        sinb = sinb.copy()
        sinb[:, 0] = np.cos(np.pi * (128 * c + np.arange(128)))   # (-1)^u
        ftc[:, 2560 + c * 256:2560 + c * 256 + 128] = cosb
        ftc[:, 2560 + c * 256 + 128:2560 + c * 256 + 256] = sinb
    return ftc, w


def _get_exec():
    """Build (once) and return a cached executor fn(in_maps) -> full output."""
    if "exec" in _STATE:
        return _STATE["exec"]

    from concourse import bass2jax

    nc = _build_nc()

    def run(in_maps):
        res = bass2jax.run_bass_via_pjrt(nc, in_maps, n_cores=NCORES)
        return np.concatenate(
            [np.asarray(r["out"]).astype(np.float32) for r in res], axis=0
        )

    _STATE["exec"] = run
    return run


def _prep_inputs(x: np.ndarray, basis: np.ndarray):
    import ml_dtypes

    xp_all = np.zeros((B, L), ml_dtypes.bfloat16)
    xp_all[:, PAD:PAD + T] = np.asarray(x, np.float32).astype(ml_dtypes.bfloat16)
    # transposed overlap layout: xst[b, p, j] = xp[b, p + 64 j]
    sb = xp_all.strides
    xst = np.ascontiguousarray(np.lib.stride_tricks.as_strided(
        xp_all, (B, 128, JC), (sb[0], sb[1], 64 * sb[1])))
    ftc, w = _host_prep_basis(basis)
    ft = ftc.astype(ml_dtypes.bfloat16)
    # window vectors as raw fp32 bits in the bf16 tensor (kernel bitcasts)
    w8 = np.stack(
        [w[128 * c:128 * c + 128] for c in range(4)]
        + [w[512 + 128 * c:512 + 128 * c + 128] for c in range(4)], axis=1
    ).astype(np.float32)                            # [128, 8]
    ft.view(np.uint16)[:, 3072:3088] = w8.view(np.uint16)
    in_maps = [
        {
            "xst": xst[BPC * c:BPC * (c + 1)],
            "ft": ft,
        }
        for c in range(NCORES)
    ]
    return in_maps


def kernel(x: np.ndarray, basis: np.ndarray) -> np.ndarray:
    run = _get_exec()
    in_maps = _prep_inputs(x, basis)
    return run(in_maps)                            # [32, 513, 501, 2]


# revision 46
# speedup vs baseline: 3.1487x; 1.0943x over previous
"""ConvSTFT kernel for Trainium2 (Bass/Tile), data-parallel over batch on 8 cores.

Math: out[b, k, f, i] = sum_n xp[b, 320 f + n] * basis[i*513 + k, n]
where xp = x padded with 512 zeros on both sides, f in [0, 501), n in [0, 1024).

Layout: Xs[p, j] = xp[p + 64 j] via the DMA XBAR transpose; contraction chunk
c of frame f is the stride-5 view Xs[:, 2c + 5f].

The 1024-point windowed DFT is factored with a 2-level decimation-in-
frequency split (twiddles folded into the second-stage matrices), computed on
the vector engines as chunkwise butterflies:

    y  = w * frame          (window, per-partition scalars)
    s  = y[:512] + y[512:]      d = y[:512] - y[512:]
    ss = s[:256] + s[256:]      sd = s[:256] - s[256:]
    odd bins 2m+1 = DFT-rows(d),  bins 4t+2 = DFT-rows(sd),
    bins 4t (incl. 512) = DFT-rows(ss)

which cuts the tensor-engine work from 64 to 24 matmuls (502 cycles each)
per batch. All data is bf16 (fp32 PSUM accumulation); output is written bf16
and widened on the host — the 2e-2 error budget is ~3x above what this
costs. Bin 512 (re) rides in the zero imag-bin-0 column of the "4t" group
and is routed out by a tiny fixup.
"""

import numpy as np
from contextlib import ExitStack

import concourse.bass as bass
import concourse.tile as tile
from concourse import bacc, mybir

# problem constants (hardcoded per harness contract)
B, T = 32, 160000
NCORES = 8
BPC = B // NCORES          # batches per core
HOP, NFFT = 320, 1024
BINS, F = 513, 501         # freq bins, frames
FP = 502                   # frames padded to even
PAD = NFFT // 2            # 512
JC = 2560                  # Xs columns (>= 2*7 + 5*(FP-1) + 1, mult of 16)
L = 127 + 64 * (JC - 1) + 1  # 163904 padded xp length
BF16 = mybir.dt.bfloat16
FTW = 3072 + 16            # 24 lhs tiles + 8 fp32 window vectors (bitcast)

_STATE: dict = {}


def _build_nc():
    nc = bacc.Bacc(
        "TRN2", target_bir_lowering=False, debug=False, num_devices=NCORES
    )
    f32 = mybir.dt.float32
    add, sub, mult = (
        mybir.AluOpType.add, mybir.AluOpType.subtract, mybir.AluOpType.mult
    )
    xst = nc.dram_tensor("xst", [BPC, 128, JC], BF16, kind="ExternalInput").ap()
    ft = nc.dram_tensor("ft", [128, FTW], BF16, kind="ExternalInput").ap()
    out = nc.dram_tensor("out", [BPC, BINS, F, 2], BF16, kind="ExternalOutput").ap()

    with tile.TileContext(nc) as tc, ExitStack() as ctx:
        const_pool = ctx.enter_context(tc.tile_pool(name="const", bufs=1))
        xs_pool = ctx.enter_context(tc.tile_pool(name="xs", bufs=4))
        bf_pool = ctx.enter_context(tc.tile_pool(name="bf", bufs=4))
        st_pool = ctx.enter_context(tc.tile_pool(name="st", bufs=4))
        sx_pool = ctx.enter_context(tc.tile_pool(name="sx", bufs=2))
        acc_pool = ctx.enter_context(tc.tile_pool(name="acc", bufs=4, space="PSUM"))
        wu_pool = ctx.enter_context(tc.tile_pool(name="wu", bufs=1))

        # PE p-state warmup: keep the tensor engine's busy-streak alive
        # through the startup DMA window so real matmuls run at full clock
        dummy = wu_pool.tile([128, 512], BF16, tag="dummy")
        nc.vector.memset(dummy[:], 0)
        pw = acc_pool.tile([128, 1024], f32, tag="acc")
        for _ in range(13):
            nc.tensor.matmul(
                pw[:, 0:512], dummy[:, 0:128], dummy[:], start=True, stop=True
            )

        # startup DMA chain (serialized, ~1.5-2us/hop): window scalars, then
        # the batch-0 xbar (gates the butterflies), then the lhs matrices in
        # consumption order
        ft_sb = const_pool.tile([128, FTW], BF16, tag="ft")
        nc.scalar.dma_start(ft_sb[:, 3072:FTW], ft[:, 3072:FTW])

        # input loads: host has already laid x out transposed (xst[b, p, j]
        # = padded x[b, p + 64 j]), so these are plain contiguous DMAs
        xs_t, xs_i = [], []
        for b in range(BPC):
            xs = xs_pool.tile([128, JC], BF16, tag="xs")
            xs_t.append(xs)
            xs_i.append(nc.sync.dma_start(xs[:], xst[b]))

        nc.scalar.dma_start(ft_sb[:, 0:2048], ft[:, 0:2048])
        nc.scalar.dma_start(ft_sb[:, 2048:3072], ft[:, 2048:3072])

        f32w = lambda o: ft_sb[:, o:o + 2].bitcast(f32)
        wl = [f32w(3072 + 2 * c) for c in range(4)]
        wh = [f32w(3080 + 2 * c) for c in range(4)]

        # stx buffers are zeroed once at startup; per-batch writes touch only
        # the even slots, the odd slots stay zero (imag of bin 512 is 0)
        stx_bufs = []
        for _ in range(2):
            sx = sx_pool.tile([1, 2 * FP], BF16, tag="stx")
            nc.gpsimd.memset(sx[:], 0)
            stx_bufs.append(sx)

        for b in range(BPC):
            xs = xs_t[b]

            def view(c):
                return xs[:, 2 * c: 2 * c + 5 * FP: 5]

            # butterflies, engine-balanced (STT has no DVE fast path, so use
            # tensor_scalar + tensor_tensor which run at 2x):
            #   u = wl*y_lo, t = wh*y_hi   (TS: DVE x7, Act x1)
            #   d = u - t, s = u + t       (TT: d + s23 on DVE, s01 on Pool)
            #   ss = s0+s2 (DVE), sd = s0-s2 (Pool)
            uv, tv = [], []
            for c in range(4):
                u_c = bf_pool.tile([128, FP], BF16, tag=f"u{c}")
                nc.vector.tensor_scalar_mul(u_c[:], view(c), wl[c])
                t_c = bf_pool.tile([128, FP], BF16, tag=f"t{c}")
                nc.vector.tensor_scalar_mul(t_c[:], view(c + 4), wh[c])
                uv.append(u_c); tv.append(t_c)
            dv, sv = [], [None] * 4
            for c in range(4):
                d_c = bf_pool.tile([128, FP], BF16, tag=f"d{c}")
                nc.vector.tensor_tensor(d_c[:], uv[c][:], tv[c][:], sub)
                dv.append(d_c)
            for c in range(4):
                s_c = bf_pool.tile([128, FP], BF16, tag=f"s{c}")
                eng = nc.gpsimd if c < 2 else nc.vector
                eng.tensor_tensor(s_c[:], uv[c][:], tv[c][:], add)
                sv[c] = s_c
            ssv, sdv = [], []
            for ch in range(2):
                ss_c = bf_pool.tile([128, FP], BF16, tag=f"ss{ch}")
                nc.vector.tensor_tensor(ss_c[:], sv[ch][:], sv[ch + 2][:], add)
                ssv.append(ss_c)
            for ch in range(2):
                sd_c = bf_pool.tile([128, FP], BF16, tag=f"sd{ch}")
                nc.gpsimd.tensor_tensor(sd_c[:], sv[ch][:], sv[ch + 2][:], sub)
                sdv.append(sd_c)

            # groups ordered by rhs readiness: d (DVE, early) -> ss (DVE) ->
            # sd (Pool, latest); (lhs base, rhs chunks, out row base, step).
            # The last batch ends on an odd group (no fixup ops in the tail).
            groups = [
                (0, dv, 1, 2, False),        # odd bins 1,3,..,255: m=0..127
                (1024, dv, 257, 2, False),   # odd bins 257,..,511: m=128..255
                (2560, ssv, 0, 4, True),     # bins 4t (+ bin 512 fixup)
                (2048, sdv, 2, 4, False),    # bins 4t+2
            ]
            for base, rhs_l, k0, kstep, is_ee in groups:
                # one 2-bank PSUM tile: re half at col 0, im half at col 512,
                # so a single multi-dim copy interleaves both into st
                ps = acc_pool.tile([128, 1024], f32, tag="acc")
                nch = len(rhs_l)
                for ci in range(nch):
                    lr = ft_sb[:, base + ci * 256: base + ci * 256 + 128]
                    li = ft_sb[:, base + ci * 256 + 128: base + ci * 256 + 256]
                    nc.tensor.matmul(
                        ps[:, 0:FP], lr, rhs_l[ci][:],
                        start=(ci == 0), stop=(ci == nch - 1)
                    )
                    nc.tensor.matmul(
                        ps[:, 512:512 + FP], li, rhs_l[ci][:],
                        start=(ci == 0), stop=(ci == nch - 1)
                    )
                if is_ee:
                    # partition 0 of the im half holds real bin 512, not the
                    # (identically zero) imag bin 0: route it out via stx,
                    # then zero the odd slots of row 0 (imag bin 0)
                    st = st_pool.tile([128, 2 * FP], BF16, tag="st")
                    st3 = st[:].rearrange("p (f i) -> p f i", i=2)
                    ps3 = ps[:].rearrange("p (i f) -> p f i", i=2)[:, 0:FP, :]
                    nc.scalar.copy(st3, ps3)
                    nc.scalar.mul(st[0:1, 1:2 * FP:2], st[0:1, 1:2 * FP:2], 0.0)
                    stx = stx_bufs[b % 2]
                    nc.scalar.copy(stx[:, 0:2 * FP:2], ps[0:1, 512:512 + FP])
                    nc.sync.dma_start(out[b, 512:513, :, :], stx[:, 0:2 * F])
                else:
                    st = st_pool.tile([128, 2 * FP], BF16, tag="st")
                    st3 = st[:].rearrange("p (f i) -> p f i", i=2)
                    ps3 = ps[:].rearrange("p (i f) -> p f i", i=2)[:, 0:FP, :]
                    if b == BPC - 1 and k0 == 2:
                        # very last group: halve the drain across DVE+Act and
                        # DMA each half as it lands to shorten the tail
                        HF = FP // 2
                        nc.vector.tensor_copy(st3[:, 0:HF, :], ps3[:, 0:HF, :])
                        nc.scalar.copy(st3[:, HF:FP, :], ps3[:, HF:FP, :])
                        dst1 = bass.AP(
                            out.tensor, (b * BINS + k0) * F * 2,
                            [[kstep * F * 2, 128], [1, 2 * HF]],
                        )
                        dst2 = bass.AP(
                            out.tensor, (b * BINS + k0) * F * 2 + 2 * HF,
                            [[kstep * F * 2, 128], [1, 2 * F - 2 * HF]],
                        )
                        nc.sync.dma_start(dst1, st[:, 0:2 * HF])
                        nc.sync.dma_start(dst2, st[:, 2 * HF:2 * F])
                        continue
                    # final batch: split drains across DVE+Act so the tail
                    # chain is half as long (DVE is idle by then)
                    if b == BPC - 1 and k0 in (257,):
                        nc.vector.tensor_copy(st3, ps3)
                    else:
                        nc.scalar.copy(st3, ps3)
                dst = bass.AP(
                    out.tensor, (b * BINS + k0) * F * 2,
                    [[kstep * F * 2, 128], [1, F * 2]],
                )
                nc.sync.dma_start(dst, st[:, 0:2 * F])

    nc.compile()
    return nc


def _host_prep_basis(basis: np.ndarray):
    """Build the 2-level DIF lhs matrices + window columns, [128, FTW]."""
    w = np.asarray(basis, np.float64)[0]            # basis row 0 = window
    ftc = np.zeros((128, FTW), np.float64)
    p = np.arange(128)[:, None]
    j = np.arange(128)[None, :]

    def dft_block(qbase, kof, kstep):
        ang = 2.0 * np.pi * ((qbase + p) * (kof + kstep * j)) / NFFT
        return np.cos(ang), -np.sin(ang)

    for g in range(2):          # odd bins 2m+1, m = 128g + j, q = 128c + p
        for c in range(4):
            cosb, sinb = dft_block(128 * c, 2 * (128 * g) + 1, 2)
            ftc[:, g * 1024 + c * 256:g * 1024 + c * 256 + 128] = cosb
            ftc[:, g * 1024 + c * 256 + 128:g * 1024 + c * 256 + 256] = sinb
    for c in range(2):          # bins 4t+2, u = 128c + p
        cosb, sinb = dft_block(128 * c, 2, 4)
        ftc[:, 2048 + c * 256:2048 + c * 256 + 128] = cosb
        ftc[:, 2048 + c * 256 + 128:2048 + c * 256 + 256] = sinb
    for c in range(2):          # bins 4t, u = 128c + p; im col 0 -> bin512 re
        cosb, sinb = dft_block(128 * c, 0, 4)
        sinb = sinb.copy()
        sinb[:, 0] = np.cos(np.pi * (128 * c + np.arange(128)))   # (-1)^u
        ftc[:, 2560 + c * 256:2560 + c * 256 + 128] = cosb
        ftc[:, 2560 + c * 256 + 128:2560 + c * 256 + 256] = sinb
    return ftc, w


def _get_exec():
    """Build (once) and return a cached executor fn(in_maps) -> full output."""
    if "exec" in _STATE:
        return _STATE["exec"]

    from concourse import bass2jax

    nc = _build_nc()

    def run(in_maps):
        res = bass2jax.run_bass_via_pjrt(nc, in_maps, n_cores=NCORES)
        return np.concatenate(
            [np.asarray(r["out"]).astype(np.float32) for r in res], axis=0
        )

    _STATE["exec"] = run
    return run


def _prep_inputs(x: np.ndarray, basis: np.ndarray):
    import ml_dtypes

    xp_all = np.zeros((B, L), ml_dtypes.bfloat16)
    xp_all[:, PAD:PAD + T] = np.asarray(x, np.float32).astype(ml_dtypes.bfloat16)
    # transposed overlap layout: xst[b, p, j] = xp[b, p + 64 j]
    sb = xp_all.strides
    xst = np.ascontiguousarray(np.lib.stride_tricks.as_strided(
        xp_all, (B, 128, JC), (sb[0], sb[1], 64 * sb[1])))
    ftc, w = _host_prep_basis(basis)
    ft = ftc.astype(ml_dtypes.bfloat16)
    # window vectors as raw fp32 bits in the bf16 tensor (kernel bitcasts)
    w8 = np.stack(
        [w[128 * c:128 * c + 128] for c in range(4)]
        + [w[512 + 128 * c:512 + 128 * c + 128] for c in range(4)], axis=1
    ).astype(np.float32)                            # [128, 8]
    ft.view(np.uint16)[:, 3072:3088] = w8.view(np.uint16)
    in_maps = [
        {
            "xst": xst[BPC * c:BPC * (c + 1)],
            "ft": ft,
        }
        for c in range(NCORES)
    ]
    return in_maps


def kernel(x: np.ndarray, basis: np.ndarray) -> np.ndarray:
    run = _get_exec()
    in_maps = _prep_inputs(x, basis)
    return run(in_maps)                            # [32, 513, 501, 2]


# revision 59
# speedup vs baseline: 3.1959x; 1.0150x over previous
"""ConvSTFT kernel for Trainium2 (Bass/Tile), data-parallel over batch on 8 cores.

Math: out[b, k, f, i] = sum_n xp[b, 320 f + n] * basis[i*513 + k, n]
where xp = x padded with 512 zeros on both sides, f in [0, 501), n in [0, 1024).

Layout: Xs[p, j] = xp[p + 64 j] via the DMA XBAR transpose; contraction chunk
c of frame f is the stride-5 view Xs[:, 2c + 5f].

The 1024-point windowed DFT is factored with a 2-level decimation-in-
frequency split (twiddles folded into the second-stage matrices), computed on
the vector engines as chunkwise butterflies:

    y  = w * frame          (window, per-partition scalars)
    s  = y[:512] + y[512:]      d = y[:512] - y[512:]
    ss = s[:256] + s[256:]      sd = s[:256] - s[256:]
    odd bins 2m+1 = DFT-rows(d),  bins 4t+2 = DFT-rows(sd),
    bins 4t (incl. 512) = DFT-rows(ss)

which cuts the tensor-engine work from 64 to 24 matmuls (502 cycles each)
per batch. All data is bf16 (fp32 PSUM accumulation); output is written bf16
and widened on the host — the 2e-2 error budget is ~3x above what this
costs. Bin 512 (re) rides in the zero imag-bin-0 column of the "4t" group
and is routed out by a tiny fixup.
"""

import numpy as np
from contextlib import ExitStack

import concourse.bass as bass
import concourse.tile as tile
from concourse import bacc, mybir

# problem constants (hardcoded per harness contract)
B, T = 32, 160000
NCORES = 8
BPC = B // NCORES          # batches per core
HOP, NFFT = 320, 1024
BINS, F = 513, 501         # freq bins, frames
FP = 502                   # frames padded to even
PAD = NFFT // 2            # 512
JC = 2560                  # Xs columns (>= 2*7 + 5*(FP-1) + 1, mult of 16)
L = 127 + 64 * (JC - 1) + 1  # 163904 padded xp length
BF16 = mybir.dt.bfloat16
FTW = 3072 + 16            # 24 lhs tiles + 8 fp32 window vectors (bitcast)

_STATE: dict = {}


def _build_nc():
    nc = bacc.Bacc(
        "TRN2", target_bir_lowering=False, debug=False, num_devices=NCORES
    )
    f32 = mybir.dt.float32
    add, sub, mult = (
        mybir.AluOpType.add, mybir.AluOpType.subtract, mybir.AluOpType.mult
    )
    xst = nc.dram_tensor("xst", [BPC, 128, JC], BF16, kind="ExternalInput").ap()
    ft = nc.dram_tensor("ft", [128, FTW], BF16, kind="ExternalInput").ap()
    out = nc.dram_tensor("out", [BPC, BINS, F, 2], BF16, kind="ExternalOutput").ap()

    with tile.TileContext(nc) as tc, ExitStack() as ctx:
        const_pool = ctx.enter_context(tc.tile_pool(name="const", bufs=1))
        xs_pool = ctx.enter_context(tc.tile_pool(name="xs", bufs=4))
        bf_pool = ctx.enter_context(tc.tile_pool(name="bf", bufs=4))
        st_pool = ctx.enter_context(tc.tile_pool(name="st", bufs=4))
        sx_pool = ctx.enter_context(tc.tile_pool(name="sx", bufs=2))
        acc_pool = ctx.enter_context(tc.tile_pool(name="acc", bufs=4, space="PSUM"))
        wu_pool = ctx.enter_context(tc.tile_pool(name="wu", bufs=1))

        # PE p-state warmup: keep the tensor engine's busy-streak alive
        # through the startup DMA window so real matmuls run at full clock
        dummy = wu_pool.tile([128, 512], BF16, tag="dummy")
        nc.vector.memset(dummy[:], 0)
        pw = acc_pool.tile([128, 1024], f32, tag="acc")
        for _ in range(14):
            nc.tensor.matmul(
                pw[:, 0:512], dummy[:, 0:128], dummy[:], start=True, stop=True
            )

        # startup DMA chain (serialized, ~1.5-2us/hop): window scalars, then
        # the batch-0 xbar (gates the butterflies), then the lhs matrices in
        # consumption order
        ft_sb = const_pool.tile([128, FTW], BF16, tag="ft")
        nc.scalar.dma_start(ft_sb[:, 3072:FTW], ft[:, 3072:FTW])

        # input loads: host has already laid x out transposed (xst[b, p, j]
        # = padded x[b, p + 64 j]), so these are plain contiguous DMAs
        xs_t, xs_i = [], []
        for b in range(BPC):
            xs = xs_pool.tile([128, JC], BF16, tag="xs")
            xs_t.append(xs)
            xs_i.append(nc.sync.dma_start(xs[:], xst[b]))

        nc.scalar.dma_start(ft_sb[:, 0:2048], ft[:, 0:2048])
        nc.scalar.dma_start(ft_sb[:, 2048:3072], ft[:, 2048:3072])

        f32w = lambda o: ft_sb[:, o:o + 2].bitcast(f32)
        wl = [f32w(3072 + 2 * c) for c in range(4)]
        wh = [f32w(3080 + 2 * c) for c in range(4)]

        # stx/stz buffers are zeroed once at startup; per-batch writes touch
        # only the even slots, the odd slots stay zero (imag of bins 0/512)
        stx_bufs, stz_bufs = [], []
        for _ in range(2):
            sx = sx_pool.tile([1, 2 * FP], BF16, tag="stx")
            nc.gpsimd.memset(sx[:], 0)
            stx_bufs.append(sx)
            sz = sx_pool.tile([1, 2 * FP], BF16, tag="stz")
            nc.gpsimd.memset(sz[:], 0)
            stz_bufs.append(sz)

        for b in range(BPC):
            xs = xs_t[b]

            def view(c):
                return xs[:, 2 * c: 2 * c + 5 * FP: 5]

            # butterflies, engine-balanced (STT has no DVE fast path, so use
            # tensor_scalar + tensor_tensor which run at 2x):
            #   u = wl*y_lo, t = wh*y_hi   (TS: DVE x7, Act x1)
            #   d = u - t, s = u + t       (TT: d + s23 on DVE, s01 on Pool)
            #   ss = s0+s2 (DVE), sd = s0-s2 (Pool)
            uv, tv = [], []
            for c in range(4):
                u_c = bf_pool.tile([128, FP], BF16, tag=f"u{c}")
                nc.vector.tensor_scalar_mul(u_c[:], view(c), wl[c])
                t_c = bf_pool.tile([128, FP], BF16, tag=f"t{c}")
                nc.vector.tensor_scalar_mul(t_c[:], view(c + 4), wh[c])
                uv.append(u_c); tv.append(t_c)
            dv, sv = [], [None] * 4
            for c in range(4):
                d_c = bf_pool.tile([128, FP], BF16, tag=f"d{c}")
                nc.vector.tensor_tensor(d_c[:], uv[c][:], tv[c][:], sub)
                dv.append(d_c)
            for c in range(4):
                s_c = bf_pool.tile([128, FP], BF16, tag=f"s{c}")
                eng = nc.gpsimd if c < 2 else nc.vector
                eng.tensor_tensor(s_c[:], uv[c][:], tv[c][:], add)
                sv[c] = s_c
            ssv, sdv = [], []
            for ch in range(2):
                ss_c = bf_pool.tile([128, FP], BF16, tag=f"ss{ch}")
                nc.vector.tensor_tensor(ss_c[:], sv[ch][:], sv[ch + 2][:], add)
                ssv.append(ss_c)
            for ch in range(2):
                sd_c = bf_pool.tile([128, FP], BF16, tag=f"sd{ch}")
                nc.gpsimd.tensor_tensor(sd_c[:], sv[ch][:], sv[ch + 2][:], sub)
                sdv.append(sd_c)

            # groups ordered by rhs readiness: d (DVE, early) -> ss (DVE) ->
            # sd (Pool, latest); (lhs base, rhs chunks, out row base, step).
            # The last batch ends on an odd group (no fixup ops in the tail).
            groups = [
                (0, dv, 1, 2, False),        # odd bins 1,3,..,255: m=0..127
                (1024, dv, 257, 2, False),   # odd bins 257,..,511: m=128..255
                (2560, ssv, 0, 4, True),     # bins 4t (+ bin 512 fixup)
                (2048, sdv, 2, 4, False),    # bins 4t+2
            ]
            for base, rhs_l, k0, kstep, is_ee in groups:
                # one 2-bank PSUM tile: re half at col 0, im half at col 512,
                # so a single multi-dim copy interleaves both into st
                ps = acc_pool.tile([128, 1024], f32, tag="acc")
                nch = len(rhs_l)
                for ci in range(nch):
                    lr = ft_sb[:, base + ci * 256: base + ci * 256 + 128]
                    li = ft_sb[:, base + ci * 256 + 128: base + ci * 256 + 256]
                    nc.tensor.matmul(
                        ps[:, 0:FP], lr, rhs_l[ci][:],
                        start=(ci == 0), stop=(ci == nch - 1)
                    )
                    nc.tensor.matmul(
                        ps[:, 512:512 + FP], li, rhs_l[ci][:],
                        start=(ci == 0), stop=(ci == nch - 1)
                    )
                if is_ee:
                    # partition 0 of the im half holds real bin 512, not the
                    # (identically zero) imag bin 0. Rows for bins 0 and 512
                    # go via the pre-zeroed stz/stx minibuffers so the main
                    # drain's DMA (rows 4..508) gates only on the drain.
                    st = st_pool.tile([128, 2 * FP], BF16, tag="st")
                    st3 = st[:].rearrange("p (f i) -> p f i", i=2)
                    ps3 = ps[:].rearrange("p (i f) -> p f i", i=2)[:, 0:FP, :]
                    nc.scalar.copy(st3, ps3)
                    stx = stx_bufs[b % 2]
                    stz = stz_bufs[b % 2]
                    if b == BPC - 1:
                        nc.vector.tensor_copy(stx[:, 0:2 * FP:2], ps[0:1, 512:512 + FP])
                        nc.vector.tensor_copy(stz[:, 0:2 * FP:2], ps[0:1, 0:FP])
                    else:
                        nc.scalar.copy(stx[:, 0:2 * FP:2], ps[0:1, 512:512 + FP])
                        nc.scalar.copy(stz[:, 0:2 * FP:2], ps[0:1, 0:FP])
                    dst_ee = bass.AP(
                        out.tensor, (b * BINS + k0 + kstep) * F * 2,
                        [[kstep * F * 2, 127], [1, F * 2]],
                    )
                    nc.sync.dma_start(dst_ee, st[1:128, 0:2 * F])
                    nc.sync.dma_start(out[b, 0:1, :, :], stz[:, 0:2 * F])
                    nc.sync.dma_start(out[b, 512:513, :, :], stx[:, 0:2 * F])
                    continue
                else:
                    st = st_pool.tile([128, 2 * FP], BF16, tag="st")
                    st3 = st[:].rearrange("p (f i) -> p f i", i=2)
                    ps3 = ps[:].rearrange("p (i f) -> p f i", i=2)[:, 0:FP, :]
                    if b == BPC - 1 and k0 == 2:
                        # very last group: halve the drain across DVE+Act and
                        # DMA each half as it lands to shorten the tail
                        HF = FP // 2
                        nc.vector.tensor_copy(st3[:, 0:HF, :], ps3[:, 0:HF, :])
                        nc.scalar.copy(st3[:, HF:FP, :], ps3[:, HF:FP, :])
                        dst1 = bass.AP(
                            out.tensor, (b * BINS + k0) * F * 2,
                            [[kstep * F * 2, 128], [1, 2 * HF]],
                        )
                        dst2 = bass.AP(
                            out.tensor, (b * BINS + k0) * F * 2 + 2 * HF,
                            [[kstep * F * 2, 128], [1, 2 * F - 2 * HF]],
                        )
                        nc.sync.dma_start(dst1, st[:, 0:2 * HF])
                        nc.scalar.dma_start(dst2, st[:, 2 * HF:2 * F])
                        continue
                    # final batch: split drains across DVE+Act so the tail
                    # chain is half as long (DVE is idle by then)
                    if b == BPC - 1 and k0 in (257,):
                        nc.vector.tensor_copy(st3, ps3)
                    else:
                        nc.scalar.copy(st3, ps3)
                dst = bass.AP(
                    out.tensor, (b * BINS + k0) * F * 2,
                    [[kstep * F * 2, 128], [1, F * 2]],
                )
                nc.sync.dma_start(dst, st[:, 0:2 * F])

    nc.compile()
    return nc


def _host_prep_basis(basis: np.ndarray):
    """Build the 2-level DIF lhs matrices + window columns, [128, FTW]."""
    w = np.asarray(basis, np.float64)[0]            # basis row 0 = window
    ftc = np.zeros((128, FTW), np.float64)
    p = np.arange(128)[:, None]
    j = np.arange(128)[None, :]

    def dft_block(qbase, kof, kstep):
        ang = 2.0 * np.pi * ((qbase + p) * (kof + kstep * j)) / NFFT
        return np.cos(ang), -np.sin(ang)

    for g in range(2):          # odd bins 2m+1, m = 128g + j, q = 128c + p
        for c in range(4):
            cosb, sinb = dft_block(128 * c, 2 * (128 * g) + 1, 2)
            ftc[:, g * 1024 + c * 256:g * 1024 + c * 256 + 128] = cosb
            ftc[:, g * 1024 + c * 256 + 128:g * 1024 + c * 256 + 256] = sinb
    for c in range(2):          # bins 4t+2, u = 128c + p
        cosb, sinb = dft_block(128 * c, 2, 4)
        ftc[:, 2048 + c * 256:2048 + c * 256 + 128] = cosb
        ftc[:, 2048 + c * 256 + 128:2048 + c * 256 + 256] = sinb
    for c in range(2):          # bins 4t, u = 128c + p; im col 0 -> bin512 re
        cosb, sinb = dft_block(128 * c, 0, 4)
        sinb = sinb.copy()
        sinb[:, 0] = np.cos(np.pi * (128 * c + np.arange(128)))   # (-1)^u
        ftc[:, 2560 + c * 256:2560 + c * 256 + 128] = cosb
        ftc[:, 2560 + c * 256 + 128:2560 + c * 256 + 256] = sinb
    return ftc, w


def _get_exec():
    """Build (once) and return a cached executor fn(in_maps) -> full output."""
    if "exec" in _STATE:
        return _STATE["exec"]

    from concourse import bass2jax

    nc = _build_nc()

    def run(in_maps):
        res = bass2jax.run_bass_via_pjrt(nc, in_maps, n_cores=NCORES)
        return np.concatenate(
            [np.asarray(r["out"]).astype(np.float32) for r in res], axis=0
        )

    _STATE["exec"] = run
    return run


def _prep_inputs(x: np.ndarray, basis: np.ndarray):
    import ml_dtypes

    xp_all = np.zeros((B, L), ml_dtypes.bfloat16)
    xp_all[:, PAD:PAD + T] = np.asarray(x, np.float32).astype(ml_dtypes.bfloat16)
    # transposed overlap layout: xst[b, p, j] = xp[b, p + 64 j]
    sb = xp_all.strides
    xst = np.ascontiguousarray(np.lib.stride_tricks.as_strided(
        xp_all, (B, 128, JC), (sb[0], sb[1], 64 * sb[1])))
    ftc, w = _host_prep_basis(basis)
    ft = ftc.astype(ml_dtypes.bfloat16)
    # window vectors as raw fp32 bits in the bf16 tensor (kernel bitcasts)
    w8 = np.stack(
        [w[128 * c:128 * c + 128] for c in range(4)]
        + [w[512 + 128 * c:512 + 128 * c + 128] for c in range(4)], axis=1
    ).astype(np.float32)                            # [128, 8]
    ft.view(np.uint16)[:, 3072:3088] = w8.view(np.uint16)
    in_maps = [
        {
            "xst": xst[BPC * c:BPC * (c + 1)],
            "ft": ft,
        }
        for c in range(NCORES)
    ]
    return in_maps


def kernel(x: np.ndarray, basis: np.ndarray) -> np.ndarray:
    run = _get_exec()
    in_maps = _prep_inputs(x, basis)
    return run(in_maps)                            # [32, 513, 501, 2]
